# revision 31
# baseline (speedup 1.0000x reference)
"""Causal attention with L2-normalized Q/K — Trainium2 Bass kernel.

Problem shapes (hardcoded): X [2, 2048, 1024], Wq/Wk/Wv [1024, 1024],
Wo [1024, 1024], bo [1024]; H=16 heads, d_head=64.

Sharding: 8 cores = 2 batches x 4 head-groups (4 heads each).
Core c handles batch b=c//4, heads 4*(c%4)..4*(c%4)+3.
Each core computes QKV projections for its head slice, per-head
normalized causal attention, and a partial output projection
V_hat @ Wo[slice]; a 4-core ReduceScatter sums the partials and
scatters rows, so core with rank r returns output rows 512r..512r+512
of its batch.

On-device layout notes:
- X is fed pre-transposed (XT [1024, 2048]) so the d_model contraction
  sits on SBUF partitions for all projections.
- Q and K are produced transposed ([j, t]); V in natural [t, j] layout
  augmented with a ones column per head (row 64 of the attention
  output accumulates the softmax denominator for free).
- Scores are computed as S^T [k, q] per head; exp() evacuates PSUM on
  the scalar engine (no max-subtraction needed: normalized q.k/8 is
  bounded by 0.125); causal masking is a multiply with a precomputed
  mask for diagonal tiles only.
- The 1e-6 in the reference's q/(|q|+eps) is dropped: |q| ~ 8, so the
  relative effect is ~1e-7, below fp32 noise.
"""

import math
import numpy as np
from contextlib import ExitStack

import concourse.bass as bass
import concourse.tile as tile
from concourse import mybir
from concourse.bass import _add_dep_helper as add_dep
from concourse.bass_utils import run_bass_kernel_spmd
from concourse.vector_clock import ScopedClock

F32 = mybir.dt.float32
AF = mybir.ActivationFunctionType

B, N, D, H, DH = 2, 2048, 1024, 16, 64
NH = 4            # heads per core
J = NH * DH       # head dims per core = 256
P = 128
NQ = 512          # q chunk (moving free dim / psum bank)
NKT = N // P      # 16 k-tiles per head
ID = D // P       # 8 i-tiles of d_model
VW = DH + 1       # 65: V columns + ones column

_MAX_WAITS = 1


def _split_excess_waits(nc, limit=_MAX_WAITS):
    """This walrus build caps sem waits per instruction (~4). Tile can
    emit more (kernel-tail Drain, collectives reading many-writer DRAM).
    Move excess waits onto injected same-engine NoOps right before the
    instruction; in-order execution preserves the semantics."""
    ctr = 0
    for fn in nc.m.functions:
        for bb in fn.blocks:
            out = []
            changed = False
            for ins in bb.instructions:
                si = ins.sync_info
                waits = list(si.on_wait) if si and si.on_wait else []
                if len(waits) > limit:
                    changed = True
                    chunks = [
                        waits[i : i + limit] for i in range(0, len(waits), limit)
                    ]
                    for ch in chunks[:-1]:
                        nop = mybir.InstNoOp(
                            name=f"I-waitsplit-{ctr}", ins=[], outs=[]
                        )
                        ctr += 1
                        nop.engine = ins.engine
                        nop.sync_info = mybir.SyncInfo(on_wait=ch, on_update=[])
                        out.append(nop)
                    ins.sync_info = mybir.SyncInfo(
                        on_wait=chunks[-1], on_update=list(si.on_update or [])
                    )
                out.append(ins)
            if changed:
                bb.instructions = out


def _build():
    nc = bass.Bass("TRN2", target_bir_lowering=False, debug=False, num_devices=8)

    xt = nc.dram_tensor("xt", [D, N], F32, kind="ExternalInput").ap()
    wq = nc.dram_tensor("wq", [D, J], F32, kind="ExternalInput").ap()
    wk = nc.dram_tensor("wk", [D, J], F32, kind="ExternalInput").ap()
    wv = nc.dram_tensor("wv", [D, J], F32, kind="ExternalInput").ap()
    wo = nc.dram_tensor("wo", [J, D], F32, kind="ExternalInput").ap()
    bias4 = nc.dram_tensor("bias4", [D], F32, kind="ExternalInput").ap()
    maskd = nc.dram_tensor("maskd", [P, 896], F32, kind="ExternalInput").ap()
    onesd = nc.dram_tensor("onesd", [P, 1], F32, kind="ExternalInput").ap()
    y_ext = nc.dram_tensor("y", [N // 4, D], F32, kind="ExternalOutput").ap()

    ypart = nc.dram_tensor("ypart", [N, D], F32)
    yrs = nc.dram_tensor("yrs", [N // 4, D], F32)
    # DRAM scratch rows for partition-broadcasts (walrus in this env
    # rejects the gpsimd partition_broadcast ucode op, and engines can't
    # read SBUF with partition-stride 0 — but DMA from DRAM can)
    nrow_d = nc.dram_tensor("nrow_d", [16, 2 * NQ], F32)
    den_d = nc.dram_tensor("den_d", [16, NQ], F32)

    with tile.TileContext(nc) as tc:
        with ExitStack() as ctx:
            sb = ctx.enter_context(tc.tile_pool(name="sb", bufs=1))
            ps = ctx.enter_context(tc.tile_pool(name="ps", bufs=1, space="PSUM"))

            # ---- loads ----
            # one tile per i-slice: a single DMA writer each keeps the
            # per-instruction sync-wait count low for consumers
            xtv = xt.rearrange("(i p) n -> i p n", p=P)
            xt_tiles = []
            for i in range(ID):
                t = sb.tile([P, N], F32, tag=f"xt{i}", name=f"xt{i}")
                nc.sync.dma_start(t[:], xtv[i])
                xt_tiles.append(t)

            def load_w(ap_in, name):
                t = sb.tile([P, ID, J], F32, tag="w3", bufs=4, name=name)
                nc.sync.dma_start(t[:], ap_in.rearrange("(i p) j -> p i j", p=P))
                return t

            wq_sb = load_w(wq, "wq_sb")
            wk_sb = load_w(wk, "wk_sb")
            wv_sb = load_w(wv, "wv_sb")

            wo_sb = sb.tile([P, 2, D], F32, tag="wo")
            nc.sync.dma_start(wo_sb[:], wo.rearrange("(j p) m -> p j m", p=P))

            bias_sb = sb.tile([P, D], F32, tag="bias")
            nc.sync.dma_start(
                bias_sb[:], bias4.rearrange("(a m) -> a m", a=1).to_broadcast((P, D))
            )
            mask_sb = sb.tile([P, 896], F32, tag="mask")
            nc.sync.dma_start(mask_sb[:], maskd)
            ones_sb = sb.tile([P, 1], F32, tag="ones")
            nc.sync.dma_start(ones_sb[:], onesd)

            # ---- projections ----
            qt_sb = sb.tile([P, 2, N], F32, tag="qt")
            kt_sb = sb.tile([P, 2, N], F32, tag="kt")

            def proj_t(w_sb, out_sb, nrow_base):
                # transposed projection with fused L2-norm reciprocal scaling
                for jt in range(2):
                    for tc4 in range(N // NQ):
                        tsl = bass.ts(tc4, NQ)
                        pp = ps.tile([P, NQ], F32, tag="big", bufs=6)
                        for i in range(ID):
                            nc.tensor.matmul(
                                pp[:],
                                lhsT=w_sb[:, i, bass.ts(jt, P)],
                                rhs=xt_tiles[i][:, tsl],
                                start=(i == 0),
                                stop=(i == ID - 1),
                            )
                        sq = sb.tile([P, NQ], F32, tag="sq", bufs=1)
                        nc.scalar.square(sq[:], pp[:])
                        su0 = ps.tile([P, NQ], F32, tag="big", bufs=6)
                        nc.tensor.matmul(
                            su0[0:1, :], lhsT=ones_sb[0:64, :], rhs=sq[0:64, :],
                            start=True, stop=True,
                        )
                        su1 = ps.tile([P, NQ], F32, tag="big", bufs=6)
                        nc.tensor.matmul(
                            su1[0:1, :], lhsT=ones_sb[64:128, :], rhs=sq[64:128, :],
                            start=True, stop=True,
                        )
                        nrm = sb.tile([1, 2 * NQ], F32, tag="nrm", bufs=2)
                        nc.scalar.activation(nrm[0:1, 0:NQ], su0[0:1, :], AF.Sqrt)
                        nc.scalar.activation(nrm[0:1, NQ:], su1[0:1, :], AF.Sqrt)
                        ridx = nrow_base + 4 * jt + tc4
                        nrow = nrow_d.ap()[ridx : ridx + 1, :]
                        nc.sync.dma_start(nrow, nrm[0:1, :])
                        rb = sb.tile([P, NQ], F32, tag="rb", bufs=2)
                        nc.sync.dma_start(
                            rb[0:64, :], nrow[:, 0:NQ].to_broadcast((64, NQ))
                        )
                        nc.sync.dma_start(
                            rb[64:128, :], nrow[:, NQ:].to_broadcast((64, NQ))
                        )
                        nc.vector.reciprocal(rb[:], rb[:])
                        nc.vector.tensor_mul(out_sb[:, jt, tsl], pp[:], rb[:])

            proj_t(wq_sb, qt_sb, 0)
            proj_t(wk_sb, kt_sb, 8)

            # V in natural layout, ones column per head
            v_sb = sb.tile([P, NKT, NH * VW], F32, tag="v")
            v4 = v_sb.rearrange("p t (h x) -> p t h x", h=NH)
            nc.gpsimd.memset(v4[:, :, :, DH : DH + 1], 1.0)
            for tt in range(NKT):
                pp = ps.tile([P, J], F32, tag="big", bufs=6)
                for i in range(ID):
                    nc.tensor.matmul(
                        pp[:],
                        lhsT=xt_tiles[i][:, bass.ts(tt, P)],
                        rhs=wv_sb[:, i, :],
                        start=(i == 0),
                        stop=(i == ID - 1),
                    )
                nc.vector.tensor_copy(
                    v4[:, tt, :, 0:DH],
                    pp[:].rearrange("p (h x) -> p h x", x=DH),
                )

            # ---- attention ----
            # vhat reuses two of the freed weight slots (tag w3)
            vhat_tiles = [
                sb.tile([P, N], F32, tag="w3", bufs=4, name=f"vhat{jt}")
                for jt in range(2)
            ]
            CH = 3
            for hp in range(2):
                for qc in range(N // NQ):
                    qsl = bass.ts(qc, NQ)
                    nkt = 4 * qc + 4
                    ots = []
                    for h01 in range(2):
                        ot = ps.tile([P, NQ], F32, tag="ot", bufs=2)
                        ots.append(ot)
                    for c0 in range(0, nkt, CH):
                        kts = range(c0, min(c0 + CH, nkt))
                        pts = {}
                        sts = {}
                        for kt in kts:
                            for h01 in range(2):
                                hsl = slice(64 * h01, 64 * h01 + 64)
                                st = ps.tile([P, NQ], F32, tag="big", bufs=6)
                                nc.tensor.matmul(
                                    st[:],
                                    lhsT=kt_sb[hsl, hp, bass.ts(kt, P)],
                                    rhs=qt_sb[hsl, hp, qsl],
                                    start=True,
                                    stop=True,
                                )
                                sts[(kt, h01)] = st
                        for kt in kts:
                            for h01 in range(2):
                                pt = sb.tile([P, NQ], F32, tag="pt", bufs=6)
                                nc.scalar.activation(
                                    pt[:], sts[(kt, h01)][:], AF.Exp,
                                    scale=1.0 / math.sqrt(DH),
                                )
                                dj = kt - 4 * qc
                                if dj >= 0:  # diagonal tile: causal mask
                                    nc.vector.tensor_mul(
                                        pt[:],
                                        pt[:],
                                        mask_sb[:, 384 - P * dj : 896 - P * dj],
                                    )
                                pts[(kt, h01)] = pt
                        for kt in kts:
                            for h01 in range(2):
                                h = 2 * hp + h01
                                nc.tensor.matmul(
                                    ots[h01][0:VW, :],
                                    lhsT=v_sb[:, kt, VW * h : VW * h + VW],
                                    rhs=pts[(kt, h01)][:],
                                    start=(kt == 0),
                                    stop=(kt == nkt - 1),
                                )
                    for h01 in range(2):
                        den = sb.tile([1, NQ], F32, tag="den", bufs=2)
                        nc.vector.reciprocal(den[0:1, :], ots[h01][DH : DH + 1, :])
                        didx = 8 * hp + 2 * qc + h01
                        drow = den_d.ap()[didx : didx + 1, :]
                        nc.sync.dma_start(drow, den[0:1, :])
                        rbo = sb.tile([64, NQ], F32, tag="rbo", bufs=2)
                        nc.sync.dma_start(
                            rbo[0:64, :], drow.to_broadcast((64, NQ))
                        )
                        nc.vector.tensor_mul(
                            vhat_tiles[hp][64 * h01 : 64 * h01 + 64, qsl],
                            ots[h01][0:DH, :],
                            rbo[0:64, :],
                        )

            # ---- output projection + bias/4 ----
            ypart_v = ypart.ap().rearrange("(t p) m -> t p m", p=P)
            ydmas = []
            for tt in range(NKT):
                for mc in range(2):
                    msl = bass.ts(mc, NQ)
                    yp = ps.tile([P, NQ], F32, tag="big", bufs=6)
                    for jt in range(2):
                        nc.tensor.matmul(
                            yp[:],
                            lhsT=vhat_tiles[jt][:, bass.ts(tt, P)],
                            rhs=wo_sb[:, jt, msl],
                            start=(jt == 0),
                            stop=(jt == 1),
                        )
                    ysb = sb.tile([P, NQ], F32, tag="w3", bufs=4)
                    nc.vector.tensor_add(ysb[:], yp[:], bias_sb[:, msl])
                    ydmas.append(
                        nc.sync.dma_start(ypart_v[tt][:, msl], ysb[:])
                    )

            # ---- reduce-scatter across the 4 cores of this batch ----
            # fan the 32 store deps into gpsimd nops (<=3 sem waits each),
            # then run the collective on the same engine (in-order, no
            # extra waits on the collective itself)
            cc = nc.gpsimd.collective_compute(
                "ReduceScatter",
                mybir.AluOpType.add,
                replica_groups=[[0, 1, 2, 3], [4, 5, 6, 7]],
                ins=[ypart.ap()],
                outs=[yrs.ap()],
            )
            outdma = nc.sync.dma_start(y_ext, yrs.ap())
            add_dep(outdma.ins, cc.ins, sync=True, reason="out after rs")

    _split_excess_waits(nc)
    return nc


_NC = None


def _get_nc():
    global _NC
    if _NC is None:
        _NC = _build()
    return _NC


def _make_mask():
    r = np.arange(P)[:, None]
    c = np.arange(896)[None, :]
    return (r <= c - 384).astype(np.float32)


def kernel(X, Wq, Wk, Wv, Wo, bo):
    X = np.asarray(X, dtype=np.float32)
    Wq = np.asarray(Wq, dtype=np.float32)
    Wk = np.asarray(Wk, dtype=np.float32)
    Wv = np.asarray(Wv, dtype=np.float32)
    Wo = np.asarray(Wo, dtype=np.float32)
    bo = np.asarray(bo, dtype=np.float32)

    nc = _get_nc()
    mask = _make_mask()
    ones = np.ones((P, 1), np.float32)
    bias4 = (bo * 0.25).astype(np.float32)
    xts = [np.ascontiguousarray(X[b].T) for b in range(B)]

    in_maps = []
    for c in range(8):
        b, g = c // 4, c % 4
        jsl = slice(g * J, (g + 1) * J)
        in_maps.append(
            {
                "xt": xts[b],
                "wq": np.ascontiguousarray(Wq[:, jsl]),
                "wk": np.ascontiguousarray(Wk[:, jsl]),
                "wv": np.ascontiguousarray(Wv[:, jsl]),
                "wo": np.ascontiguousarray(Wo[jsl, :]),
                "bias4": bias4,
                "maskd": mask,
                "onesd": ones,
            }
        )

    res = run_bass_kernel_spmd(nc, in_maps, list(range(8)))
    out = np.empty((B, N, D), np.float32)
    for c in range(8):
        b, r = c // 4, c % 4
        out[b, r * (N // 4) : (r + 1) * (N // 4), :] = res.results[c]["y"]
    return out


# revision 33
# speedup vs baseline: 1.1147x; 1.1147x over previous
"""Causal attention with L2-normalized Q/K — Trainium2 Bass kernel.

Problem shapes (hardcoded): X [2, 2048, 1024], Wq/Wk/Wv [1024, 1024],
Wo [1024, 1024], bo [1024]; H=16 heads, d_head=64.

Sharding: 8 cores = 2 batches x 4 head-groups (4 heads each).
Core c handles batch b=c//4, heads 4*(c%4)..4*(c%4)+3.
Each core computes QKV projections for its head slice, per-head
normalized causal attention, and a partial output projection
V_hat @ Wo[slice]. The partials are summed with per-q-chunk
ReduceScatters across the 4 cores of the batch (pipelined with
compute); core rank r returns rows 128*qc + ... strips that the host
reassembles.

Layout notes:
- X is fed pre-transposed (XT [1024, 2048]) so the d_model contraction
  sits on SBUF partitions for all projections.
- Q and K are produced transposed ([j, t]); V in natural [t, j] layout
  augmented with a ones column per head (row 64 of the attention
  output accumulates the softmax denominator for free).
- Scores are computed as S^T [k, q] per head; exp() evacuates PSUM on
  the scalar engine (no max-subtraction needed: normalized q.k/8 is
  bounded by 0.125). Causal masking: fully-masked column spans are
  memset to zero (exp skipped), the diagonal 128x128 block gets a
  triangular mask multiply.
- The 1e-6 in the reference's q/(|q|+eps) is dropped: |q| ~ 8, so the
  relative effect is ~1e-7, below fp32 noise.
- Norm reciprocals are broadcast across partitions via a DRAM
  round-trip (DMA from DRAM may read with partition-stride 0; the
  gpsimd partition_broadcast ucode op doesn't compile in this env).
"""

import math
import numpy as np
from contextlib import ExitStack

import concourse.bass as bass
import concourse.tile as tile
from concourse import mybir
from concourse.bass import _add_dep_helper as add_dep
from concourse.bass_utils import run_bass_kernel_spmd

F32 = mybir.dt.float32
AF = mybir.ActivationFunctionType

B, N, D, H, DH = 2, 2048, 1024, 16, 64
NH = 4            # heads per core
J = NH * DH       # head dims per core = 256
P = 128
NQ = 512          # q chunk (moving free dim / psum bank)
NKT = N // P      # 16 k-tiles per head
ID = D // P       # 8 i-tiles of d_model
VW = DH + 1       # 65: V columns + ones column
NQC = N // NQ     # 4 q-chunks

_MAX_WAITS = 1


def _split_excess_waits(nc, limit=_MAX_WAITS):
    """This walrus build allows very few sem waits per instruction.
    Tile can emit many (kernel-tail Drain, collectives reading
    many-writer DRAM). Move excess waits onto injected same-engine
    NoOps right before the instruction; in-order execution preserves
    the semantics."""
    ctr = 0
    for fn in nc.m.functions:
        for bb in fn.blocks:
            out = []
            changed = False
            for ins in bb.instructions:
                si = ins.sync_info
                waits = list(si.on_wait) if si and si.on_wait else []
                if len(waits) > limit:
                    changed = True
                    chunks = [
                        waits[i : i + limit] for i in range(0, len(waits), limit)
                    ]
                    for ch in chunks[:-1]:
                        nop = mybir.InstNoOp(
                            name=f"I-waitsplit-{ctr}", ins=[], outs=[]
                        )
                        ctr += 1
                        nop.engine = ins.engine
                        nop.sync_info = mybir.SyncInfo(on_wait=ch, on_update=[])
                        out.append(nop)
                    ins.sync_info = mybir.SyncInfo(
                        on_wait=chunks[-1], on_update=list(si.on_update or [])
                    )
                out.append(ins)
            if changed:
                bb.instructions = out


def _build():
    nc = bass.Bass("TRN2", target_bir_lowering=False, debug=False, num_devices=8)

    xt = nc.dram_tensor("xt", [D, N], F32, kind="ExternalInput").ap()
    wq = nc.dram_tensor("wq", [D, J], F32, kind="ExternalInput").ap()
    wk = nc.dram_tensor("wk", [D, J], F32, kind="ExternalInput").ap()
    wv = nc.dram_tensor("wv", [D, J], F32, kind="ExternalInput").ap()
    wo = nc.dram_tensor("wo", [J, D], F32, kind="ExternalInput").ap()
    bias4 = nc.dram_tensor("bias4", [D], F32, kind="ExternalInput").ap()
    maskd = nc.dram_tensor("maskd", [P, P], F32, kind="ExternalInput").ap()
    onesd = nc.dram_tensor("onesd", [P, 1], F32, kind="ExternalInput").ap()
    # output: 4 strips of [128, D], strip qc = rows 512*qc+128*rank of
    # this batch's final output (host reassembles)
    y_ext = nc.dram_tensor("y", [N // 4, D], F32, kind="ExternalOutput").ap()

    # per-q-chunk partial/reduced buffers so each ReduceScatter only
    # depends on its own chunk's stores
    yparts = [nc.dram_tensor(f"ypart{qc}", [NQ, D], F32) for qc in range(NQC)]
    yrss = [nc.dram_tensor(f"yrs{qc}", [P, D], F32) for qc in range(NQC)]
    # DRAM scratch rows for partition-broadcast round-trips
    nrow_d = nc.dram_tensor("nrow_d", [16, 2 * NQ], F32)
    den_d = nc.dram_tensor("den_d", [16, NQ], F32)

    with tile.TileContext(nc) as tc:
        with ExitStack() as ctx:
            sb = ctx.enter_context(tc.tile_pool(name="sb", bufs=1))
            ps = ctx.enter_context(tc.tile_pool(name="ps", bufs=1, space="PSUM"))

            # ---- loads ----
            # xt as an 8x4 grid of [128, 512] tiles: single-writer tiles
            # keep consumer sync-wait counts low, and quarter-granular
            # arrival lets the first projection chunk start early.
            xtv = xt.rearrange("(i p) n -> i p n", p=P)
            xt_tiles = [[None] * 4 for _ in range(ID)]

            def load_xt_quarter(i, c):
                t = sb.tile([P, NQ], F32, tag=f"x{i}c{c}", name=f"x{i}c{c}")
                nc.sync.dma_start(t[:], xtv[i][:, bass.ts(c, NQ)])
                xt_tiles[i][c] = t

            def load_w_half(ap_in, nm, h):
                t = sb.tile([P, 4, J], F32, tag=f"{nm}{h}", name=f"{nm}{h}")
                v = ap_in.rearrange("(i p) j -> i p j", p=P)
                nc.sync.dma_start(
                    t[:], v[4 * h : 4 * h + 4].rearrange("i p j -> p i j")
                )
                return t

            # emission order = queue fill order: first-needed first
            wq_h = [load_w_half(wq, "wq", h) for h in range(2)]
            for i in range(ID):
                load_xt_quarter(i, 0)
            for i in range(ID):
                load_xt_quarter(i, 1)
            wk_h = [load_w_half(wk, "wk", h) for h in range(2)]
            for i in range(ID):
                load_xt_quarter(i, 2)
            for i in range(ID):
                load_xt_quarter(i, 3)
            wv_h = [load_w_half(wv, "wv", h) for h in range(2)]

            wo_sb = sb.tile([P, 2, D], F32, tag="wo")
            nc.sync.dma_start(wo_sb[:], wo.rearrange("(j p) m -> p j m", p=P))
            bias_sb = sb.tile([P, D], F32, tag="bias")
            nc.sync.dma_start(
                bias_sb[:], bias4.rearrange("(a m) -> a m", a=1).to_broadcast((P, D))
            )
            mask_sb = sb.tile([P, P], F32, tag="mask")
            nc.sync.dma_start(mask_sb[:], maskd)
            ones_sb = sb.tile([P, 1], F32, tag="ones")
            nc.sync.dma_start(ones_sb[:], onesd)

            # ---- projections (QT/KT transposed + normalized; V natural) ----
            qt_sb = sb.tile([P, 2, N], F32, tag="qt")
            kt_sb = sb.tile([P, 2, N], F32, tag="kt")

            def proj_t(w_h, out_sb, nrow_base):
                for jt in range(2):
                    for tc4 in range(NQC):
                        tsl = bass.ts(tc4, NQ)
                        pp = ps.tile([P, NQ], F32, tag="big", bufs=5)
                        for i in range(ID):
                            nc.tensor.matmul(
                                pp[:],
                                lhsT=w_h[i // 4][:, i % 4, bass.ts(jt, P)],
                                rhs=xt_tiles[i][tc4][:],
                                start=(i == 0),
                                stop=(i == ID - 1),
                            )
                        sq = sb.tile([P, NQ], F32, tag="sq", bufs=1)
                        nc.scalar.square(sq[:], pp[:])
                        su0 = ps.tile([P, NQ], F32, tag="big", bufs=5)
                        nc.tensor.matmul(
                            su0[0:1, :], lhsT=ones_sb[0:64, :], rhs=sq[0:64, :],
                            start=True, stop=True,
                        )
                        su1 = ps.tile([P, NQ], F32, tag="big", bufs=5)
                        nc.tensor.matmul(
                            su1[0:1, :], lhsT=ones_sb[64:128, :], rhs=sq[64:128, :],
                            start=True, stop=True,
                        )
                        nrm = sb.tile([1, 2 * NQ], F32, tag="nrm", bufs=1)
                        nc.scalar.activation(nrm[0:1, 0:NQ], su0[0:1, :], AF.Sqrt)
                        nc.scalar.activation(nrm[0:1, NQ:], su1[0:1, :], AF.Sqrt)
                        ridx = nrow_base + 4 * jt + tc4
                        nrow = nrow_d.ap()[ridx : ridx + 1, :]
                        nc.sync.dma_start(nrow, nrm[0:1, :])
                        rb = sb.tile([P, NQ], F32, tag="rb", bufs=2)
                        nc.sync.dma_start(
                            rb[0:64, :], nrow[:, 0:NQ].to_broadcast((64, NQ))
                        )
                        nc.sync.dma_start(
                            rb[64:128, :], nrow[:, NQ:].to_broadcast((64, NQ))
                        )
                        nc.vector.reciprocal(rb[:], rb[:])
                        nc.vector.tensor_mul(out_sb[:, jt, tsl], pp[:], rb[:])

            proj_t(wq_h, qt_sb, 0)
            proj_t(wk_h, kt_sb, 8)

            # V natural layout, ones column per head
            v_sb = sb.tile([P, NKT, NH * VW], F32, tag="v")
            v4 = v_sb.rearrange("p t (h x) -> p t h x", h=NH)
            nc.gpsimd.memset(v4[:, :, :, DH : DH + 1], 1.0)
            for tt in range(NKT):
                pp = ps.tile([P, J], F32, tag="big", bufs=5)
                for i in range(ID):
                    nc.tensor.matmul(
                        pp[:],
                        lhsT=xt_tiles[i][tt // 4][:, bass.ts(tt % 4, P)],
                        rhs=wv_h[i // 4][:, i % 4, :],
                        start=(i == 0),
                        stop=(i == ID - 1),
                    )
                nc.vector.tensor_copy(
                    v4[:, tt, :, 0:DH],
                    pp[:].rearrange("p (h x) -> p h x", x=DH),
                )

            # ---- attention + pipelined output projection ----
            # vhat quarters reuse xt grid slots (projections done by then)
            vhat_q = {
                (jt, qc): sb.tile(
                    [P, NQ], F32, tag=f"x{4 * jt + qc}c3", name=f"vhat{jt}_{qc}"
                )
                for jt in range(2)
                for qc in range(NQC)
            }

            CH = 3
            ccs = []
            for qc in range(NQC):
                qsl = bass.ts(qc, NQ)
                nkt = 4 * qc + 4
                for hp in range(2):
                    ots = [
                        ps.tile([P, NQ], F32, tag="ot", bufs=3, name=f"ot{i}")
                        for i in range(2)
                    ]
                    for c0 in range(0, nkt, CH):
                        kts = range(c0, min(c0 + CH, nkt))
                        pts = {}
                        sts = {}
                        for kt in kts:
                            for h01 in range(2):
                                hsl = slice(64 * h01, 64 * h01 + 64)
                                st = ps.tile([P, NQ], F32, tag="big", bufs=5)
                                nc.tensor.matmul(
                                    st[:],
                                    lhsT=kt_sb[hsl, hp, bass.ts(kt, P)],
                                    rhs=qt_sb[hsl, hp, qsl],
                                    start=True,
                                    stop=True,
                                )
                                sts[(kt, h01)] = st
                        for kt in kts:
                            dj = kt - 4 * qc  # >=0: diagonal-crossing tile
                            for h01 in range(2):
                                pt = sb.tile([P, NQ], F32, tag="pt", bufs=6)
                                if dj >= 1:
                                    # columns < 128*dj fully causal-masked
                                    nc.gpsimd.memset(pt[:, 0 : P * dj], 0.0)
                                    nc.scalar.activation(
                                        pt[:, P * dj :],
                                        sts[(kt, h01)][:, P * dj :],
                                        AF.Exp,
                                        scale=1.0 / math.sqrt(DH),
                                    )
                                else:
                                    nc.scalar.activation(
                                        pt[:], sts[(kt, h01)][:], AF.Exp,
                                        scale=1.0 / math.sqrt(DH),
                                    )
                                if dj >= 0:
                                    # triangular mask on the 128-wide
                                    # diagonal block
                                    blk = slice(P * dj, P * dj + P)
                                    nc.vector.tensor_mul(
                                        pt[:, blk], pt[:, blk], mask_sb[:]
                                    )
                                pts[(kt, h01)] = pt
                        for kt in kts:
                            for h01 in range(2):
                                h = 2 * hp + h01
                                nc.tensor.matmul(
                                    ots[h01][0:VW, :],
                                    lhsT=v_sb[:, kt, VW * h : VW * h + VW],
                                    rhs=pts[(kt, h01)][:],
                                    start=(kt == 0),
                                    stop=(kt == nkt - 1),
                                )
                    for h01 in range(2):
                        den = sb.tile([1, NQ], F32, tag="den", bufs=2)
                        nc.vector.reciprocal(den[0:1, :], ots[h01][DH : DH + 1, :])
                        didx = 8 * hp + 2 * qc + h01
                        drow = den_d.ap()[didx : didx + 1, :]
                        nc.sync.dma_start(drow, den[0:1, :])
                        rbo = sb.tile([P, NQ], F32, tag=f"x{6 + h01}c2", bufs=1)
                        nc.sync.dma_start(rbo[0:64, :], drow.to_broadcast((64, NQ)))
                        nc.vector.tensor_mul(
                            vhat_q[(hp, qc)][64 * h01 : 64 * h01 + 64, :],
                            ots[h01][0:DH, :],
                            rbo[0:64, :],
                        )

                # output projection for this q-chunk + chunk ReduceScatter
                ypv = yparts[qc].ap().rearrange("(t p) m -> t p m", p=P)
                for t4 in range(4):
                    for mc in range(2):
                        msl = bass.ts(mc, NQ)
                        yp = ps.tile([P, NQ], F32, tag="big", bufs=5)
                        for jt in range(2):
                            nc.tensor.matmul(
                                yp[:],
                                lhsT=vhat_q[(jt, qc)][:, bass.ts(t4, P)],
                                rhs=wo_sb[:, jt, msl],
                                start=(jt == 0),
                                stop=(jt == 1),
                            )
                        ysb = sb.tile(
                            [P, NQ], F32, tag=f"x{(2 * t4 + mc) % 6}c2", bufs=1
                        )
                        nc.vector.tensor_add(ysb[:], yp[:], bias_sb[:, msl])
                        nc.sync.dma_start(ypv[t4][:, msl], ysb[:])

                cc = nc.gpsimd.collective_compute(
                    "ReduceScatter",
                    mybir.AluOpType.add,
                    replica_groups=[[0, 1, 2, 3], [4, 5, 6, 7]],
                    ins=[yparts[qc].ap()],
                    outs=[yrss[qc].ap()],
                )
                ccs.append(cc)
                outdma = nc.sync.dma_start(
                    y_ext[bass.ts(qc, P), :], yrss[qc].ap()
                )
                add_dep(outdma.ins, cc.ins, sync=True, reason="out after rs")

    _split_excess_waits(nc)
    return nc


_NC = None


def _get_nc():
    global _NC
    if _NC is None:
        _NC = _build()
    return _NC


def _make_mask():
    r = np.arange(P)[:, None]
    c = np.arange(P)[None, :]
    return (r <= c).astype(np.float32)


def kernel(X, Wq, Wk, Wv, Wo, bo):
    X = np.asarray(X, dtype=np.float32)
    Wq = np.asarray(Wq, dtype=np.float32)
    Wk = np.asarray(Wk, dtype=np.float32)
    Wv = np.asarray(Wv, dtype=np.float32)
    Wo = np.asarray(Wo, dtype=np.float32)
    bo = np.asarray(bo, dtype=np.float32)

    nc = _get_nc()
    mask = _make_mask()
    ones = np.ones((P, 1), np.float32)
    bias4 = (bo * 0.25).astype(np.float32)
    xts = [np.ascontiguousarray(X[b].T) for b in range(B)]

    in_maps = []
    for c in range(8):
        b, g = c // 4, c % 4
        jsl = slice(g * J, (g + 1) * J)
        in_maps.append(
            {
                "xt": xts[b],
                "wq": np.ascontiguousarray(Wq[:, jsl]),
                "wk": np.ascontiguousarray(Wk[:, jsl]),
                "wv": np.ascontiguousarray(Wv[:, jsl]),
                "wo": np.ascontiguousarray(Wo[jsl, :]),
                "bias4": bias4,
                "maskd": mask,
                "onesd": ones,
            }
        )

    res = run_bass_kernel_spmd(nc, in_maps, list(range(8)))
    out = np.empty((B, N, D), np.float32)
    for c in range(8):
        b, r = c // 4, c % 4
        yc = res.results[c]["y"]  # [512, D]: strip qc at rows 128*qc
        for qc in range(NQC):
            out[b, NQ * qc + P * r : NQ * qc + P * r + P, :] = yc[
                P * qc : P * qc + P, :
            ]
    return out


# revision 37
# speedup vs baseline: 1.5794x; 1.4169x over previous
"""Causal attention with L2-normalized Q/K — Trainium2 Bass kernel.

Problem shapes (hardcoded): X [2, 2048, 1024], Wq/Wk/Wv [1024, 1024],
Wo [1024, 1024], bo [1024]; H=16 heads, d_head=64.

Sharding: 8 cores = 2 batches x 4 head-groups (4 heads each).
Core c handles batch b=c//4, heads 4*(c%4)..4*(c%4)+3.
Each core computes QKV projections for its head slice, per-head
normalized causal attention, and a partial output projection
V_hat @ Wo[slice]. The partials are summed with per-q-chunk
ReduceScatters across the 4 cores of the batch (pipelined with
compute); core rank r returns rows 128*qc + ... strips that the host
reassembles.

Layout notes:
- X is fed pre-transposed (XT [1024, 2048]) so the d_model contraction
  sits on SBUF partitions for all projections.
- Q and K are produced transposed ([j, t]); V in natural [t, j] layout
  augmented with a ones column per head (row 64 of the attention
  output accumulates the softmax denominator for free).
- Scores are computed as S^T [k, q] per head; exp() evacuates PSUM on
  the scalar engine (no max-subtraction needed: normalized q.k/8 is
  bounded by 0.125). Causal masking: fully-masked column spans are
  memset to zero (exp skipped), the diagonal 128x128 block gets a
  triangular mask multiply.
- The 1e-6 in the reference's q/(|q|+eps) is dropped: |q| ~ 8, so the
  relative effect is ~1e-7, below fp32 noise.
- Norm reciprocals are broadcast across partitions via a DRAM
  round-trip (DMA from DRAM may read with partition-stride 0; the
  gpsimd partition_broadcast ucode op doesn't compile in this env).
"""

import math
import numpy as np
from contextlib import ExitStack

import concourse.bass as bass
import concourse.tile as tile
from concourse import mybir
from concourse.bass import _add_dep_helper as add_dep
from concourse.bass_utils import run_bass_kernel_spmd

F32 = mybir.dt.float32
F32R = mybir.dt.float32r  # TF32-like single-pass matmul dtype (~2e-4 rel)
AF = mybir.ActivationFunctionType

B, N, D, H, DH = 2, 2048, 1024, 16, 64
NH = 4            # heads per core
J = NH * DH       # head dims per core = 256
P = 128
NQ = 512          # q chunk (moving free dim / psum bank)
NKT = N // P      # 16 k-tiles per head
ID = D // P       # 8 i-tiles of d_model
VW = DH + 1       # 65: V columns + ones column
NQC = N // NQ     # 4 q-chunks

_MAX_WAITS = 1


def _split_excess_waits(nc, limit=_MAX_WAITS):
    """This walrus build allows very few sem waits per instruction.
    Tile can emit many (kernel-tail Drain, collectives reading
    many-writer DRAM). Move excess waits onto injected same-engine
    NoOps right before the instruction; in-order execution preserves
    the semantics."""
    ctr = 0
    for fn in nc.m.functions:
        for bb in fn.blocks:
            out = []
            changed = False
            for ins in bb.instructions:
                si = ins.sync_info
                waits = list(si.on_wait) if si and si.on_wait else []
                if len(waits) > limit:
                    changed = True
                    chunks = [
                        waits[i : i + limit] for i in range(0, len(waits), limit)
                    ]
                    for ch in chunks[:-1]:
                        nop = mybir.InstNoOp(
                            name=f"I-waitsplit-{ctr}", ins=[], outs=[]
                        )
                        ctr += 1
                        nop.engine = ins.engine
                        nop.sync_info = mybir.SyncInfo(on_wait=ch, on_update=[])
                        out.append(nop)
                    ins.sync_info = mybir.SyncInfo(
                        on_wait=chunks[-1], on_update=list(si.on_update or [])
                    )
                out.append(ins)
            if changed:
                bb.instructions = out


def _build():
    nc = bass.Bass("TRN2", target_bir_lowering=False, debug=False, num_devices=8)

    xt = nc.dram_tensor("xt", [D, N], F32R, kind="ExternalInput").ap()
    wq = nc.dram_tensor("wq", [D, J], F32R, kind="ExternalInput").ap()
    wk = nc.dram_tensor("wk", [D, J], F32R, kind="ExternalInput").ap()
    wv = nc.dram_tensor("wv", [D, J], F32R, kind="ExternalInput").ap()
    wo = nc.dram_tensor("wo", [J, D], F32, kind="ExternalInput").ap()
    bias4 = nc.dram_tensor("bias4", [D], F32, kind="ExternalInput").ap()
    maskd = nc.dram_tensor("maskd", [P, P], F32R, kind="ExternalInput").ap()
    onesd = nc.dram_tensor("onesd", [P, 1], F32R, kind="ExternalInput").ap()
    # output: 4 strips of [128, D], strip qc = rows 512*qc+128*rank of
    # this batch's final output (host reassembles)
    y_ext = nc.dram_tensor("y", [N // 4, D], F32, kind="ExternalOutput").ap()

    # per-q-chunk partial/reduced buffers so each ReduceScatter only
    # depends on its own chunk's stores
    yparts = [nc.dram_tensor(f"ypart{qc}", [NQ, D], F32) for qc in range(NQC)]
    yrss = [nc.dram_tensor(f"yrs{qc}", [P, D], F32) for qc in range(NQC)]
    # DRAM scratch rows for partition-broadcast round-trips
    nrow_d = nc.dram_tensor("nrow_d", [16, 2 * NQ], F32)
    den_d = nc.dram_tensor("den_d", [16, NQ], F32)

    with tile.TileContext(nc) as tc:
        with ExitStack() as ctx:
            sb = ctx.enter_context(tc.tile_pool(name="sb", bufs=1))
            ps = ctx.enter_context(tc.tile_pool(name="ps", bufs=1, space="PSUM"))

            # ---- loads ----
            # xt as an 8x4 grid of [128, 512] tiles: single-writer tiles
            # keep consumer sync-wait counts low, and quarter-granular
            # arrival lets the first projection chunk start early.
            xtv = xt.rearrange("(i p) n -> i p n", p=P)
            xt_tiles = [[None] * 4 for _ in range(ID)]

            def load_xt_quarter(i, c):
                t = sb.tile([P, NQ], F32R, tag=f"x{i}c{c}", name=f"x{i}c{c}")
                nc.sync.dma_start(t[:], xtv[i][:, bass.ts(c, NQ)])
                xt_tiles[i][c] = t

            def load_w_half(ap_in, nm, h):
                t = sb.tile([P, 4, J], F32R, tag=f"{nm}{h}", name=f"{nm}{h}")
                v = ap_in.rearrange("(i p) j -> i p j", p=P)
                nc.sync.dma_start(
                    t[:], v[4 * h : 4 * h + 4].rearrange("i p j -> p i j")
                )
                return t

            # emission order = queue fill order: first-needed first
            wq_h = [load_w_half(wq, "wq", h) for h in range(2)]
            for i in range(ID):
                load_xt_quarter(i, 0)
            for i in range(ID):
                load_xt_quarter(i, 1)
            wk_h = [load_w_half(wk, "wk", h) for h in range(2)]
            for i in range(ID):
                load_xt_quarter(i, 2)
            for i in range(ID):
                load_xt_quarter(i, 3)
            wv_h = [load_w_half(wv, "wv", h) for h in range(2)]

            wo_sb = sb.tile([P, 2, D], F32, tag="wo")
            nc.sync.dma_start(wo_sb[:], wo.rearrange("(j p) m -> p j m", p=P))
            bias_sb = sb.tile([P, D], F32, tag="bias")
            nc.sync.dma_start(
                bias_sb[:], bias4.rearrange("(a m) -> a m", a=1).to_broadcast((P, D))
            )
            mask_sb = sb.tile([P, P], F32R, tag="mask")
            nc.sync.dma_start(mask_sb[:], maskd)
            ones_sb = sb.tile([P, 1], F32R, tag="ones")
            nc.sync.dma_start(ones_sb[:], onesd)

            # ---- projections (QT/KT transposed + normalized; V natural) ----
            qt_sb = sb.tile([P, 2, N], F32R, tag="qt")
            kt_sb = sb.tile([P, 2, N], F32R, tag="kt")

            def proj_t(w_h, out_sb, nrow_base):
                for jt in range(2):
                    for tc4 in range(NQC):
                        tsl = bass.ts(tc4, NQ)
                        pp = ps.tile([P, NQ], F32, tag="big", bufs=5)
                        for i in range(ID):
                            nc.tensor.matmul(
                                pp[:],
                                lhsT=w_h[i // 4][:, i % 4, bass.ts(jt, P)],
                                rhs=xt_tiles[i][tc4][:],
                                start=(i == 0),
                                stop=(i == ID - 1),
                            )
                        sq = sb.tile([P, NQ], F32R, tag="sq", bufs=1)
                        nc.scalar.square(sq[:], pp[:])
                        su0 = ps.tile([P, NQ], F32, tag="big", bufs=5)
                        nc.tensor.matmul(
                            su0[0:1, :], lhsT=ones_sb[0:64, :], rhs=sq[0:64, :],
                            start=True, stop=True,
                        )
                        su1 = ps.tile([P, NQ], F32, tag="big", bufs=5)
                        nc.tensor.matmul(
                            su1[0:1, :], lhsT=ones_sb[64:128, :], rhs=sq[64:128, :],
                            start=True, stop=True,
                        )
                        nrm = sb.tile([1, 2 * NQ], F32, tag="nrm", bufs=1)
                        nc.scalar.activation(nrm[0:1, 0:NQ], su0[0:1, :], AF.Sqrt)
                        nc.scalar.activation(nrm[0:1, NQ:], su1[0:1, :], AF.Sqrt)
                        ridx = nrow_base + 4 * jt + tc4
                        nrow = nrow_d.ap()[ridx : ridx + 1, :]
                        nc.gpsimd.dma_start(nrow, nrm[0:1, :])
                        rb = sb.tile([P, NQ], F32, tag="rb", bufs=2)
                        nc.gpsimd.dma_start(
                            rb[0:64, :], nrow[:, 0:NQ].to_broadcast((64, NQ))
                        )
                        nc.gpsimd.dma_start(
                            rb[64:128, :], nrow[:, NQ:].to_broadcast((64, NQ))
                        )
                        nc.vector.reciprocal(rb[:], rb[:])
                        nc.vector.tensor_mul(out_sb[:, jt, tsl], pp[:], rb[:])

            proj_t(wq_h, qt_sb, 0)
            proj_t(wk_h, kt_sb, 8)

            # V natural layout, ones column per head
            v_sb = sb.tile([P, NKT, NH * VW], F32R, tag="v")
            v4 = v_sb.rearrange("p t (h x) -> p t h x", h=NH)
            # ones columns via broadcast-DMA (memset rejects f32r tiles)
            nc.sync.dma_start(
                v_sb.rearrange("p t (h x) -> p (t h) x", h=NH)[:, :, DH : DH + 1],
                onesd.rearrange("p (a b) -> p a b", a=1).to_broadcast(
                    (P, NKT * NH, 1)
                ),
            )
            for tt in range(NKT):
                pp = ps.tile([P, J], F32, tag="big", bufs=5)
                for i in range(ID):
                    nc.tensor.matmul(
                        pp[:],
                        lhsT=xt_tiles[i][tt // 4][:, bass.ts(tt % 4, P)],
                        rhs=wv_h[i // 4][:, i % 4, :],
                        start=(i == 0),
                        stop=(i == ID - 1),
                    )
                nc.vector.tensor_copy(
                    v4[:, tt, :, 0:DH],
                    pp[:].rearrange("p (h x) -> p h x", x=DH),
                )

            # ---- attention + pipelined output projection ----
            # vhat quarters reuse xt grid slots (projections done by then)
            vhat_q = {
                (jt, qc): sb.tile(
                    [P, NQ], F32, tag=f"x{4 * jt + qc}c3", name=f"vhat{jt}_{qc}"
                )
                for jt in range(2)
                for qc in range(NQC)
            }

            CH = 3
            ccs = []
            for qc in range(NQC):
                qsl = bass.ts(qc, NQ)
                nkt = 4 * qc + 4
                for hp in range(2):
                    ots = [
                        ps.tile([P, NQ], F32, tag="ot", bufs=3, name=f"ot{i}")
                        for i in range(2)
                    ]
                    for c0 in range(0, nkt, CH):
                        kts = range(c0, min(c0 + CH, nkt))
                        pts = {}
                        sts = {}
                        for kt in kts:
                            for h01 in range(2):
                                hsl = slice(64 * h01, 64 * h01 + 64)
                                st = ps.tile([P, NQ], F32, tag="big", bufs=5)
                                nc.tensor.matmul(
                                    st[:],
                                    lhsT=kt_sb[hsl, hp, bass.ts(kt, P)],
                                    rhs=qt_sb[hsl, hp, qsl],
                                    start=True,
                                    stop=True,
                                )
                                sts[(kt, h01)] = st
                        for kt in kts:
                            dj = kt - 4 * qc  # >=0: diagonal-crossing tile
                            for h01 in range(2):
                                pt = sb.tile([P, NQ], F32R, tag="pt", bufs=6)
                                if dj >= 1:
                                    # columns < 128*dj fully causal-masked
                                    # (x*0 instead of memset: memset can't
                                    # write f32r tiles)
                                    nc.vector.tensor_scalar_mul(
                                        pt[:, 0 : P * dj],
                                        sts[(kt, h01)][:, 0 : P * dj],
                                        0.0,
                                    )
                                    nc.scalar.activation(
                                        pt[:, P * dj :],
                                        sts[(kt, h01)][:, P * dj :],
                                        AF.Exp,
                                        scale=1.0 / math.sqrt(DH),
                                    )
                                else:
                                    nc.scalar.activation(
                                        pt[:], sts[(kt, h01)][:], AF.Exp,
                                        scale=1.0 / math.sqrt(DH),
                                    )
                                if dj >= 0:
                                    # triangular mask on the 128-wide
                                    # diagonal block
                                    blk = slice(P * dj, P * dj + P)
                                    nc.vector.tensor_mul(
                                        pt[:, blk], pt[:, blk], mask_sb[:]
                                    )
                                pts[(kt, h01)] = pt
                        for kt in kts:
                            for h01 in range(2):
                                h = 2 * hp + h01
                                nc.tensor.matmul(
                                    ots[h01][0:VW, :],
                                    lhsT=v_sb[:, kt, VW * h : VW * h + VW],
                                    rhs=pts[(kt, h01)][:],
                                    start=(kt == 0),
                                    stop=(kt == nkt - 1),
                                )
                    for h01 in range(2):
                        den = sb.tile([1, NQ], F32, tag="den", bufs=2)
                        nc.vector.reciprocal(den[0:1, :], ots[h01][DH : DH + 1, :])
                        didx = 8 * hp + 2 * qc + h01
                        drow = den_d.ap()[didx : didx + 1, :]
                        nc.gpsimd.dma_start(drow, den[0:1, :])
                        rbo = sb.tile([P, NQ], F32, tag=f"x{6 + h01}c2", bufs=1)
                        nc.gpsimd.dma_start(rbo[0:64, :], drow.to_broadcast((64, NQ)))
                        nc.vector.tensor_mul(
                            vhat_q[(hp, qc)][64 * h01 : 64 * h01 + 64, :],
                            ots[h01][0:DH, :],
                            rbo[0:64, :],
                        )

                # output projection for this q-chunk + chunk ReduceScatter
                ypv = yparts[qc].ap().rearrange("(t p) m -> t p m", p=P)
                for t4 in range(4):
                    for mc in range(2):
                        msl = bass.ts(mc, NQ)
                        yp = ps.tile([P, NQ], F32, tag="big", bufs=5)
                        for jt in range(2):
                            nc.tensor.matmul(
                                yp[:],
                                lhsT=vhat_q[(jt, qc)][:, bass.ts(t4, P)],
                                rhs=wo_sb[:, jt, msl],
                                start=(jt == 0),
                                stop=(jt == 1),
                            )
                        ysb = sb.tile(
                            [P, NQ], F32, tag=f"x{(2 * t4 + mc) % 6}c2", bufs=1
                        )
                        nc.vector.tensor_add(ysb[:], yp[:], bias_sb[:, msl])
                        nc.sync.dma_start(ypv[t4][:, msl], ysb[:])

                cc = nc.gpsimd.collective_compute(
                    "ReduceScatter",
                    mybir.AluOpType.add,
                    replica_groups=[[0, 1, 2, 3], [4, 5, 6, 7]],
                    ins=[yparts[qc].ap()],
                    outs=[yrss[qc].ap()],
                )
                ccs.append(cc)
                outdma = nc.sync.dma_start(
                    y_ext[bass.ts(qc, P), :], yrss[qc].ap()
                )
                add_dep(outdma.ins, cc.ins, sync=True, reason="out after rs")

    _split_excess_waits(nc)
    return nc


_NC = None


def _get_nc():
    global _NC
    if _NC is None:
        _NC = _build()
    return _NC


def _make_mask():
    r = np.arange(P)[:, None]
    c = np.arange(P)[None, :]
    return (r <= c).astype(np.float32)


def kernel(X, Wq, Wk, Wv, Wo, bo):
    X = np.asarray(X, dtype=np.float32)
    Wq = np.asarray(Wq, dtype=np.float32)
    Wk = np.asarray(Wk, dtype=np.float32)
    Wv = np.asarray(Wv, dtype=np.float32)
    Wo = np.asarray(Wo, dtype=np.float32)
    bo = np.asarray(bo, dtype=np.float32)

    nc = _get_nc()
    mask = _make_mask()
    ones = np.ones((P, 1), np.float32)
    bias4 = (bo * 0.25).astype(np.float32)
    xts = [np.ascontiguousarray(X[b].T) for b in range(B)]

    in_maps = []
    for c in range(8):
        b, g = c // 4, c % 4
        jsl = slice(g * J, (g + 1) * J)
        in_maps.append(
            {
                "xt": xts[b],
                "wq": np.ascontiguousarray(Wq[:, jsl]),
                "wk": np.ascontiguousarray(Wk[:, jsl]),
                "wv": np.ascontiguousarray(Wv[:, jsl]),
                "wo": np.ascontiguousarray(Wo[jsl, :]),
                "bias4": bias4,
                "maskd": mask,
                "onesd": ones,
            }
        )

    res = run_bass_kernel_spmd(nc, in_maps, list(range(8)))
    out = np.empty((B, N, D), np.float32)
    for c in range(8):
        b, r = c // 4, c % 4
        yc = res.results[c]["y"]  # [512, D]: strip qc at rows 128*qc
        for qc in range(NQC):
            out[b, NQ * qc + P * r : NQ * qc + P * r + P, :] = yc[
                P * qc : P * qc + P, :
            ]
    return out


# revision 40
# speedup vs baseline: 1.6531x; 1.0467x over previous
"""Causal attention with L2-normalized Q/K — Trainium2 Bass kernel.

Problem shapes (hardcoded): X [2, 2048, 1024], Wq/Wk/Wv [1024, 1024],
Wo [1024, 1024], bo [1024]; H=16 heads, d_head=64.

Sharding: 8 cores = 2 batches x 4 head-groups (4 heads each).
Core c handles batch b=c//4, heads 4*(c%4)..4*(c%4)+3.
Each core computes QKV projections for its head slice, per-head
normalized causal attention, and a partial output projection
V_hat @ Wo[slice]. The partials are summed with per-q-chunk
ReduceScatters across the 4 cores of the batch (pipelined with
compute); the host reassembles the row strips.

Layout notes:
- X is fed pre-transposed and pre-tiled ([i, c, 128, 512] blocks) so
  every load is one large contiguous descriptor and the d_model
  contraction sits on SBUF partitions for all projections.
- Q and K are produced transposed ([j, t]) in per-chunk tiles; V in
  natural [t, j] layout augmented with a ones column per head (row 64
  of the attention output accumulates the softmax denominator free).
- Matmul operands use float32r (TF32-like, single-pass: 2x faster
  than fp32's two-pass split; ~2e-4 rel error). The output projection
  stays fp32.
- Scores are computed as S^T [k, q] per head; exp() evacuates PSUM on
  the scalar engine (no max-subtraction needed: normalized q.k/8 is
  bounded by 0.125). Causal masking: fully-masked column spans are
  zeroed (exp skipped), the diagonal 128x128 block gets a triangular
  mask multiply.
- The 1e-6 in the reference's q/(|q|+eps) is dropped: |q| ~ 8, so the
  relative effect is ~1e-7, below fp32 noise.
- Norm/denominator reciprocals use reciprocal_approx_fast (~4e-6 rel,
  5x faster than the stock DVE reciprocal) and are broadcast across
  partitions via a DRAM round-trip on the scalar engine's DMA queues
  (DMA from DRAM may read with partition-stride 0; gpsimd
  partition_broadcast doesn't compile in this env, and the gpsimd
  SWDGE path serializes on one queue).
"""

import math
import numpy as np
from contextlib import ExitStack

import concourse.bass as bass
import concourse.tile as tile
from concourse import mybir
from concourse.bass import _add_dep_helper as add_dep
from concourse.bass_utils import run_bass_kernel_spmd

F32 = mybir.dt.float32
F32R = mybir.dt.float32r  # TF32-like single-pass matmul dtype (~2e-4 rel)
AF = mybir.ActivationFunctionType

B, N, D, H, DH = 2, 2048, 1024, 16, 64
NH = 4            # heads per core
J = NH * DH       # head dims per core = 256
P = 128
NQ = 512          # q chunk (moving free dim / psum bank)
NKT = N // P      # 16 k-tiles per head
ID = D // P       # 8 i-tiles of d_model
VW = DH + 1       # 65: V columns + ones column
NQC = N // NQ     # 4 q-chunks

_MAX_WAITS = 1


def _split_excess_waits(nc, limit=_MAX_WAITS):
    """This walrus build allows very few sem waits per instruction.
    Tile can emit many (kernel-tail Drain, collectives reading
    many-writer DRAM). Move excess waits onto injected same-engine
    NoOps right before the instruction; in-order execution preserves
    the semantics."""
    ctr = 0
    for fn in nc.m.functions:
        for bb in fn.blocks:
            out = []
            changed = False
            for ins in bb.instructions:
                si = ins.sync_info
                waits = list(si.on_wait) if si and si.on_wait else []
                if len(waits) > limit:
                    changed = True
                    chunks = [
                        waits[i : i + limit] for i in range(0, len(waits), limit)
                    ]
                    for ch in chunks[:-1]:
                        nop = mybir.InstNoOp(
                            name=f"I-waitsplit-{ctr}", ins=[], outs=[]
                        )
                        ctr += 1
                        nop.engine = ins.engine
                        nop.sync_info = mybir.SyncInfo(on_wait=ch, on_update=[])
                        out.append(nop)
                    ins.sync_info = mybir.SyncInfo(
                        on_wait=chunks[-1], on_update=list(si.on_update or [])
                    )
                out.append(ins)
            if changed:
                bb.instructions = out


def _build():
    nc = bass.Bass("TRN2", target_bir_lowering=False, debug=False, num_devices=8)

    xt = nc.dram_tensor("xt", [ID, NQC, P, NQ], F32R, kind="ExternalInput").ap()
    wq = nc.dram_tensor("wq", [2, P, 4, J], F32R, kind="ExternalInput").ap()
    wk = nc.dram_tensor("wk", [2, P, 4, J], F32R, kind="ExternalInput").ap()
    wv = nc.dram_tensor("wv", [2, P, 4, J], F32R, kind="ExternalInput").ap()
    wo = nc.dram_tensor("wo", [P, 2, D], F32, kind="ExternalInput").ap()
    bias4 = nc.dram_tensor("bias4", [D], F32, kind="ExternalInput").ap()
    maskd = nc.dram_tensor("maskd", [P, P], F32R, kind="ExternalInput").ap()
    onesd = nc.dram_tensor("onesd", [P, 1], F32R, kind="ExternalInput").ap()
    # output: 4 strips of [128, D]; strip qc = rows 512*qc + 128*rank
    # of this batch's final output (host reassembles)
    y_ext = nc.dram_tensor("y", [N // 4, D], F32, kind="ExternalOutput").ap()

    # per-q-chunk partial/reduced buffers so each ReduceScatter only
    # depends on its own chunk's stores
    yparts = [nc.dram_tensor(f"ypart{qc}", [NQ, D], F32) for qc in range(NQC)]
    yrss = [nc.dram_tensor(f"yrs{qc}", [P, D], F32) for qc in range(NQC)]
    # DRAM scratch rows for partition-broadcast round-trips
    nrow_d = nc.dram_tensor("nrow_d", [16, 2 * NQ], F32)
    den_d = nc.dram_tensor("den_d", [16, NQ], F32)

    with tile.TileContext(nc) as tc:
        with ExitStack() as ctx:
            sb = ctx.enter_context(tc.tile_pool(name="sb", bufs=1))
            ps = ctx.enter_context(tc.tile_pool(name="ps", bufs=1, space="PSUM"))

            # ---- loads (each one contiguous DRAM block) ----
            def load_w_half(ap_in, nm, h):
                t = sb.tile([P, 4, J], F32R, tag=f"{nm}{h}", name=f"{nm}{h}")
                nc.sync.dma_start(t[:], ap_in[h])
                return t

            wq_h = [load_w_half(wq, "wq", h) for h in range(2)]
            wk_h = [load_w_half(wk, "wk", h) for h in range(2)]
            wv_h = [load_w_half(wv, "wv", h) for h in range(2)]
            xt_tiles = [[None] * NQC for _ in range(ID)]
            for c in range(NQC):
                for i in range(ID):
                    t = sb.tile([P, NQ], F32R, tag=f"x{i}c{c}", name=f"x{i}c{c}")
                    nc.sync.dma_start(t[:], xt[i, c])
                    xt_tiles[i][c] = t

            wo_sb = sb.tile([P, 2, D], F32, tag="wo")
            nc.sync.dma_start(wo_sb[:], wo)
            bias_sb = sb.tile([P, D], F32, tag="bias")
            nc.sync.dma_start(
                bias_sb[:], bias4.rearrange("(a m) -> a m", a=1).to_broadcast((P, D))
            )
            mask_sb = sb.tile([P, P], F32R, tag="mask")
            nc.sync.dma_start(mask_sb[:], maskd)
            ones_sb = sb.tile([P, 1], F32R, tag="ones")
            nc.sync.dma_start(ones_sb[:], onesd)

            # ---- projections, quarter-major so compute tracks arrival ----
            qt_t = {}
            kt_t = {}
            v_sb = sb.tile([P, NKT, NH * VW], F32R, tag="v")
            v4 = v_sb.rearrange("p t (h x) -> p t h x", h=NH)
            # ones columns via broadcast-DMA (memset rejects f32r tiles)
            nc.sync.dma_start(
                v_sb.rearrange("p t (h x) -> p (t h) x", h=NH)[:, :, DH : DH + 1],
                onesd.rearrange("p (a b) -> p a b", a=1).to_broadcast(
                    (P, NKT * NH, 1)
                ),
            )

            def proj_chunk(w_h, jt, tc4, out_t, ridx):
                # transposed projection chunk with fused L2-norm recip scale
                pp = ps.tile([P, NQ], F32, tag="big", bufs=5)
                for i in range(ID):
                    nc.tensor.matmul(
                        pp[:],
                        lhsT=w_h[i // 4][:, i % 4, bass.ts(jt, P)],
                        rhs=xt_tiles[i][tc4][:],
                        start=(i == 0),
                        stop=(i == ID - 1),
                    )
                sq = sb.tile([P, NQ], F32R, tag="sq", bufs=2)
                nc.scalar.square(sq[:], pp[:])
                su0 = ps.tile([P, NQ], F32, tag="big", bufs=5)
                nc.tensor.matmul(
                    su0[0:1, :], lhsT=ones_sb[0:64, :], rhs=sq[0:64, :],
                    start=True, stop=True,
                )
                su1 = ps.tile([P, NQ], F32, tag="big", bufs=5)
                nc.tensor.matmul(
                    su1[0:1, :], lhsT=ones_sb[64:128, :], rhs=sq[64:128, :],
                    start=True, stop=True,
                )
                # 1/(sqrt(x)) = exp(-0.5*ln(x)): stays in the Exp/Ln ACT
                # table set (no set switching, no slow DVE reciprocal)
                lnr = sb.tile([1, 2 * NQ], F32, tag="lnr", bufs=2)
                nc.scalar.activation(lnr[0:1, 0:NQ], su0[0:1, :], AF.Ln)
                nc.scalar.activation(lnr[0:1, NQ:], su1[0:1, :], AF.Ln)
                nrm = sb.tile([1, 2 * NQ], F32, tag="nrm", bufs=2)
                nc.scalar.activation(nrm[0:1, :], lnr[0:1, :], AF.Exp, scale=-0.5)
                nrow = nrow_d.ap()[ridx : ridx + 1, :]
                nc.scalar.dma_start(nrow, nrm[0:1, :])
                rb = sb.tile([P, NQ], F32, tag="rb", bufs=2)
                nc.scalar.dma_start(
                    rb[0:64, :], nrow[:, 0:NQ].to_broadcast((64, NQ))
                )
                nc.scalar.dma_start(
                    rb[64:128, :], nrow[:, NQ:].to_broadcast((64, NQ))
                )
                nc.vector.tensor_mul(out_t[:], pp[:], rb[:])

            for c in range(NQC):
                for jt in range(2):
                    qt_t[(jt, c)] = sb.tile(
                        [P, NQ], F32R, tag=f"qt{jt}{c}", name=f"qt{jt}{c}"
                    )
                    proj_chunk(wq_h, jt, c, qt_t[(jt, c)], 4 * jt + c)
                for jt in range(2):
                    kt_t[(jt, c)] = sb.tile(
                        [P, NQ], F32R, tag=f"kt{jt}{c}", name=f"kt{jt}{c}"
                    )
                    proj_chunk(wk_h, jt, c, kt_t[(jt, c)], 8 + 4 * jt + c)
                for tt in range(4 * c, 4 * c + 4):
                    pp = ps.tile([P, J], F32, tag="big", bufs=5)
                    for i in range(ID):
                        nc.tensor.matmul(
                            pp[:],
                            lhsT=xt_tiles[i][c][:, bass.ts(tt % 4, P)],
                            rhs=wv_h[i // 4][:, i % 4, :],
                            start=(i == 0),
                            stop=(i == ID - 1),
                        )
                    nc.vector.tensor_copy(
                        v4[:, tt, :, 0:DH],
                        pp[:].rearrange("p (h x) -> p h x", x=DH),
                    )

            # ---- attention + pipelined output projection ----
            # vhat quarters reuse xt grid slots (projections done by then)
            vhat_q = {
                (jt, qc): sb.tile(
                    [P, NQ], F32, tag=f"x{4 * jt + qc}c3", name=f"vhat{jt}_{qc}"
                )
                for jt in range(2)
                for qc in range(NQC)
            }

            CH = 3
            for qc in range(NQC):
                nkt = 4 * qc + 4
                for hp in range(2):
                    ots = [
                        ps.tile([P, NQ], F32, tag="ot", bufs=3, name=f"ot{i}")
                        for i in range(2)
                    ]
                    for c0 in range(0, nkt, CH):
                        kts = range(c0, min(c0 + CH, nkt))
                        pts = {}
                        sts = {}
                        for kt in kts:
                            for h01 in range(2):
                                hsl = slice(64 * h01, 64 * h01 + 64)
                                st = ps.tile([P, NQ], F32, tag="big", bufs=5)
                                nc.tensor.matmul(
                                    st[:],
                                    lhsT=kt_t[(hp, kt // 4)][
                                        hsl, bass.ts(kt % 4, P)
                                    ],
                                    rhs=qt_t[(hp, qc)][hsl, :],
                                    start=True,
                                    stop=True,
                                )
                                sts[(kt, h01)] = st
                        for kt in kts:
                            dj = kt - 4 * qc  # >=0: diagonal-crossing tile
                            for h01 in range(2):
                                pt = sb.tile([P, NQ], F32R, tag="pt", bufs=6)
                                if dj >= 1:
                                    # cols < 128*dj fully causal-masked
                                    nc.vector.tensor_scalar_mul(
                                        pt[:, 0 : P * dj],
                                        sts[(kt, h01)][:, 0 : P * dj],
                                        0.0,
                                    )
                                    nc.scalar.activation(
                                        pt[:, P * dj :],
                                        sts[(kt, h01)][:, P * dj :],
                                        AF.Exp,
                                        scale=1.0 / math.sqrt(DH),
                                    )
                                else:
                                    nc.scalar.activation(
                                        pt[:], sts[(kt, h01)][:], AF.Exp,
                                        scale=1.0 / math.sqrt(DH),
                                    )
                                if dj >= 0:
                                    blk = slice(P * dj, P * dj + P)
                                    nc.vector.tensor_mul(
                                        pt[:, blk], pt[:, blk], mask_sb[:]
                                    )
                                pts[(kt, h01)] = pt
                        for kt in kts:
                            for h01 in range(2):
                                h = 2 * hp + h01
                                nc.tensor.matmul(
                                    ots[h01][0:VW, :],
                                    lhsT=v_sb[:, kt, VW * h : VW * h + VW],
                                    rhs=pts[(kt, h01)][:],
                                    start=(kt == 0),
                                    stop=(kt == nkt - 1),
                                )
                    for h01 in range(2):
                        # 1/x = exp(-ln(x)) on ACT (same table set as exp)
                        dln = sb.tile([1, NQ], F32, tag="dln", bufs=2)
                        nc.scalar.activation(
                            dln[0:1, :], ots[h01][DH : DH + 1, :], AF.Ln
                        )
                        den = sb.tile([1, NQ], F32, tag="den", bufs=2)
                        nc.scalar.activation(
                            den[0:1, :], dln[0:1, :], AF.Exp, scale=-1.0
                        )
                        didx = 8 * hp + 2 * qc + h01
                        drow = den_d.ap()[didx : didx + 1, :]
                        nc.scalar.dma_start(drow, den[0:1, :])
                        rbo = sb.tile([P, NQ], F32, tag=f"x{6 + h01}c2", bufs=1)
                        nc.scalar.dma_start(
                            rbo[0:64, :], drow.to_broadcast((64, NQ))
                        )
                        nc.vector.tensor_mul(
                            vhat_q[(hp, qc)][64 * h01 : 64 * h01 + 64, :],
                            ots[h01][0:DH, :],
                            rbo[0:64, :],
                        )

                # output projection for this q-chunk + chunk ReduceScatter
                ypv = yparts[qc].ap().rearrange("(t p) m -> t p m", p=P)
                for t4 in range(4):
                    for mc in range(2):
                        msl = bass.ts(mc, NQ)
                        yp = ps.tile([P, NQ], F32, tag="big", bufs=5)
                        for jt in range(2):
                            nc.tensor.matmul(
                                yp[:],
                                lhsT=vhat_q[(jt, qc)][:, bass.ts(t4, P)],
                                rhs=wo_sb[:, jt, msl],
                                start=(jt == 0),
                                stop=(jt == 1),
                            )
                        ysb = sb.tile(
                            [P, NQ], F32, tag=f"x{(2 * t4 + mc) % 6}c2", bufs=1
                        )
                        nc.vector.tensor_add(ysb[:], yp[:], bias_sb[:, msl])
                        nc.sync.dma_start(ypv[t4][:, msl], ysb[:])

                cc = nc.gpsimd.collective_compute(
                    "ReduceScatter",
                    mybir.AluOpType.add,
                    replica_groups=[[0, 1, 2, 3], [4, 5, 6, 7]],
                    ins=[yparts[qc].ap()],
                    outs=[yrss[qc].ap()],
                )
                outdma = nc.sync.dma_start(
                    y_ext[bass.ts(qc, P), :], yrss[qc].ap()
                )
                add_dep(outdma.ins, cc.ins, sync=True, reason="out after rs")

    _split_excess_waits(nc)
    return nc


_NC = None


def _get_nc():
    global _NC
    if _NC is None:
        _NC = _build()
    return _NC


def _make_mask():
    r = np.arange(P)[:, None]
    c = np.arange(P)[None, :]
    return (r <= c).astype(np.float32)


def kernel(X, Wq, Wk, Wv, Wo, bo):
    X = np.asarray(X, dtype=np.float32)
    Wq = np.asarray(Wq, dtype=np.float32)
    Wk = np.asarray(Wk, dtype=np.float32)
    Wv = np.asarray(Wv, dtype=np.float32)
    Wo = np.asarray(Wo, dtype=np.float32)
    bo = np.asarray(bo, dtype=np.float32)

    nc = _get_nc()
    mask = _make_mask()
    ones = np.ones((P, 1), np.float32)
    bias4 = (bo * 0.25).astype(np.float32)
    # pre-tiled XT: [i, c, 128, 512] contiguous blocks of X[b].T
    xts = [
        np.ascontiguousarray(
            X[b].T.reshape(ID, P, NQC, NQ).transpose(0, 2, 1, 3)
        )
        for b in range(B)
    ]

    def wslice(W, jsl):
        # [1024, 256] -> [2, 128, 4, 256] half-major contiguous blocks
        return np.ascontiguousarray(
            W[:, jsl].reshape(2, 4, P, J).transpose(0, 2, 1, 3)
        )

    in_maps = []
    for c in range(8):
        b, g = c // 4, c % 4
        jsl = slice(g * J, (g + 1) * J)
        in_maps.append(
            {
                "xt": xts[b],
                "wq": wslice(Wq, jsl),
                "wk": wslice(Wk, jsl),
                "wv": wslice(Wv, jsl),
                "wo": np.ascontiguousarray(
                    Wo[jsl, :].reshape(2, P, D).transpose(1, 0, 2)
                ),
                "bias4": bias4,
                "maskd": mask,
                "onesd": ones,
            }
        )

    res = run_bass_kernel_spmd(nc, in_maps, list(range(8)))
    out = np.empty((B, N, D), np.float32)
    for c in range(8):
        b, r = c // 4, c % 4
        yc = res.results[c]["y"]  # [512, D]: strip qc at rows 128*qc
        for qc in range(NQC):
            out[b, NQ * qc + P * r : NQ * qc + P * r + P, :] = yc[
                P * qc : P * qc + P, :
            ]
    return out
